# revision 33
# baseline (speedup 1.0000x reference)
"""Cross-attention kernel for Trainium2, data-parallel over batch on 8 NeuronCores.

Per core (batch element b):
  out[b] = softmax((x[b] @ Wq.T + bq) @ (c[b] @ Wk.T + bk).T / 32) @ (c[b] @ Wv.T + bv)

Fast path (used whenever bq == 0, which includes the reference inputs):
fold M = Wq.T @ Wk on the host (weights-only transform), so
scores = x @ M @ c.T and the entire k projection disappears from the
device. bk is always softmax-invariant (it shifts every score row by a
constant), so it drops exactly. PE work: 786k matmul rows vs 917k.

Device layout (all matmul operands bf16, fp32 accumulation):
  phase 1: XM[e,s] = (M.T @ x.T)/32  (chunk-outer so the first matmul needs
           only 1MB landed), V[t,e] = c @ Wv.T + bv
  phase 2: per 128-row tile of s: S = XM.T @ CT (ct is the moving operand
           directly), P = exp(S) + row sums via ACT, per-512-chunk eager
           128x128 xbar transposes, O = P @ V, scaled by 1/rowsum on drain;
           final store split across engines/queues.
  S(0) is emitted between the xm and v projections so the attention
  pipeline starts without waiting on v psum drains.

DMA model (measured): queue arbitration is per-packet round-robin, so
concurrent queues share HBM bandwidth in proportion to their line size;
a single HW queue with >=4KB lines sustains ~300-420GB/s. The whole
phase-1 load therefore runs on the sync queue alone, strictly in
consumption order, with host-pre-tiled layouts giving 4-16KB lines.
A general fallback (build_nc_general) keeps the explicit q/k projections
for nonzero bq.
"""

import numpy as np
import ml_dtypes

import concourse.bass as bass
import concourse.mybir as mybir
import concourse.tile as tile
from concourse import bacc
from concourse.bass_utils import run_bass_kernel_spmd

DIM = 1024
SEQ = 2048
B = 8
P = 128
DT = DIM // P        # 8 contraction tiles of 128
ST = SEQ // P        # 16 seq tiles of 128
KC = SEQ // 512      # 4 key chunks of 512
EC = DIM // 512      # 2 embed chunks of 512
F32 = mybir.dt.float32
BF16 = mybir.dt.bfloat16

_CACHED_NC = None
_CACHED_NC_GENERAL = None


def build_nc():
    nc = bacc.Bacc(None, target_bir_lowering=False)

    # cc: [p, sc, dt, s_lo] = c[sc*512+s_lo, dt*128+p]  (bf16, 8KB lines)
    # xc: flat [p, 16384]; two 256-col chunks first (4KB lines) so the first
    # xm matmul needs only 0.5MB of x, then three 512-col chunks (8KB lines).
    xc = nc.declare_dram_parameter("xc", [P, KC * DT * 512], BF16, isOutput=False)
    cc = nc.declare_dram_parameter("cc", [P, KC, DT, 512], BF16, isOutput=False)
    # mt: [p, et*1024 + dt*128 + j] = M[dt*128+p, et*128+j], M = Wq.T @ Wk
    mt = nc.declare_dram_parameter("mt", [P, DT * DT * P], BF16, isOutput=False)
    # wv: [p, dt*1024 + e] = Wv[e, dt*128+p]  (16KB lines)
    wvt = nc.declare_dram_parameter("wvt", [P, DT * DIM], BF16, isOutput=False)
    bvb = nc.declare_dram_parameter("bvb", [P, DIM], F32, isOutput=False)
    out = nc.declare_dram_parameter("out", [SEQ, DIM], F32, isOutput=True)

    out_r = out.rearrange("(t p) e -> p t e", p=P)

    with tile.TileContext(nc) as tc:
        with (
            tc.tile_pool(name="res", bufs=1) as res,
            tc.tile_pool(name="psum", bufs=2, space="PSUM") as psum,
            tc.tile_pool(name="projp", bufs=3, space="PSUM") as projp,
        ):
            ppool = projp
            spsum = opsum = psum
            qt_sb = res.tile([P, DT, SEQ], BF16, tag="qt")   # xm.T / 32
            v_sb = res.tile([P, ST, DIM], BF16, tag="v")

            bv_sb = res.tile([P, DIM], F32, tag="bv")
            ct_sb = res.tile([P, KC, DT, 512], BF16, tag="ct")
            wv_sb = res.tile([P, DT * DIM], BF16, tag="wv")

            # ---------------- phase 1: xm projection ----------------
            with (
                tc.tile_pool(name="acts", bufs=1) as acts,
                tc.tile_pool(name="warmps", bufs=1, space="PSUM") as warmps,
            ):
                wqpool = acts
                # Dummy matmuls on a zeroed tile keep the PE busy through the
                # input-DMA window so HAM never sees an idle->busy transition
                # (which costs a ~7-10us half-clock window).
                wsrc = acts.tile([P, 512], BF16, tag="warm")
                nc.vector.memset(wsrc, 0.0)
                wps = warmps.tile([P, 512], F32, tag="wps")
                NWARM = 12
                for i in range(NWARM):
                    nc.tensor.matmul(
                        wps, wsrc[:, 0:P], wsrc, start=(i == 0), stop=(i == NWARM - 1)
                    )

                xt_sb = acts.tile([P, KC * DT * 512], BF16, tag="xt")
                m_sb = wqpool.tile([P, DT * DT * P], BF16, tag="m")

                nc.sync.dma_start(out=m_sb[:, 0:2048], in_=mt[:, 0:2048])
                nc.sync.dma_start(out=xt_sb[:, 0:2048], in_=xc[:, 0:2048])
                nc.sync.dma_start(out=m_sb[:, 2048:4096], in_=mt[:, 2048:4096])
                nc.sync.dma_start(out=m_sb[:, 4096:6144], in_=mt[:, 4096:6144])
                nc.sync.dma_start(out=m_sb[:, 6144:8192], in_=mt[:, 6144:8192])
                nc.sync.dma_start(out=xt_sb[:, 2048:4096], in_=xc[:, 2048:4096])
                nc.sync.dma_start(out=xt_sb[:, 4096:8192], in_=xc[:, 4096:8192])
                nc.sync.dma_start(out=xt_sb[:, 8192:12288], in_=xc[:, 8192:12288])
                nc.sync.dma_start(out=xt_sb[:, 12288:16384], in_=xc[:, 12288:16384])
                nc.sync.dma_start(out=ct_sb, in_=cc[:, :])
                nc.sync.dma_start(out=wv_sb, in_=wvt[:, :])
                nc.sync.dma_start(out=bv_sb, in_=bvb[:, :])

                # xm projection, chunk-outer in DMA-arrival order.
                XCHUNKS = [(0, 256, 0), (2048, 256, 256), (4096, 512, 512),
                           (8192, 512, 1024), (12288, 512, 1536)]
                for xoff, w, s0 in XCHUNKS:
                    for et in range(DT):
                        ps = ppool.tile([P, 512], F32, tag="proj")
                        for dt in range(DT):
                            off = et * DT * P + dt * P
                            nc.tensor.matmul(
                                ps[:, 0:w],
                                m_sb[:, off : off + P],
                                xt_sb[:, xoff + dt * w : xoff + (dt + 1) * w],
                                start=(dt == 0),
                                stop=(dt == DT - 1),
                            )
                        nc.scalar.activation(
                            out=qt_sb[:, et, s0 : s0 + w],
                            in_=ps[:, 0:w],
                            func=mybir.ActivationFunctionType.Identity,
                            scale=1.0 / 32.0,
                        )

            # ---------------- phase 2: attention ----------------
            # Software-pipelined: S/exp/transpose for tile st is emitted
            # before O/store for tile st-1. S(0) runs right after the xm
            # projection; the v projection overlaps S(0)'s exp/transposes.
            attn_cm = tc.tile_pool(name="attn", bufs=3)
            attn = attn_cm.__enter__()
            stats = attn

            def emit_s_stage(st):
                p_sb = attn.tile([P, SEQ], BF16, tag="p")
                pt_sb = attn.tile([P, ST, P], BF16, tag="pt")
                sums = stats.tile([P, KC], F32, tag="sums")
                for kc in range(KC):
                    sp = spsum.tile([P, 512], F32, tag="s")
                    for dt in range(DT):
                        nc.tensor.matmul(
                            sp,
                            qt_sb[:, dt, st * P : (st + 1) * P],
                            ct_sb[:, kc, dt, :],
                            start=(dt == 0),
                            stop=(dt == DT - 1),
                        )
                    nc.scalar.activation(
                        out=p_sb[:, kc * 512 : (kc + 1) * 512],
                        in_=sp,
                        func=mybir.ActivationFunctionType.Exp,
                        accum_out=sums[:, kc : kc + 1],
                    )
                    # eager xbar transpose of the 4 just-exp'd 128x128
                    # blocks: pt[p, tt, f] = p_sb[f, tt*128 + p]
                    nc.sync.dma_start_transpose(
                        out=pt_sb[:, kc * 4 : (kc + 1) * 4],
                        in_=p_sb[:, kc * 512 : (kc + 1) * 512],
                    )
                ssum = stats.tile([P, 1], F32, tag="ssum")
                rsum = stats.tile([P, 1], F32, tag="rsum")
                nc.vector.reduce_sum(out=ssum, in_=sums, axis=mybir.AxisListType.X)
                nc.vector.reciprocal(out=rsum, in_=ssum)
                return pt_sb, rsum

            def emit_o_stage(st, pt_sb, rsum):
                last = st == ST - 1
                o_sb = attn.tile([P, DIM], F32, tag="o")
                for ec in range(EC):
                    op = opsum.tile([P, 512], F32, tag="o")
                    for tt in range(ST):
                        nc.tensor.matmul(
                            op,
                            pt_sb[:, tt],
                            v_sb[:, tt, ec * 512 : (ec + 1) * 512],
                            start=(tt == 0),
                            stop=(tt == ST - 1),
                        )
                    if not last or ec == 0:
                        nc.vector.tensor_scalar_mul(
                            out=o_sb[:, ec * 512 : (ec + 1) * 512],
                            in0=op,
                            scalar1=rsum,
                        )
                        nc.gpsimd.dma_start(
                            out=out_r[:, st, ec * 512 : (ec + 1) * 512],
                            in_=o_sb[:, ec * 512 : (ec + 1) * 512],
                        )
                    else:
                        # final drain is latency-critical: scale the two
                        # 256-col halves on vector and scalar in parallel,
                        # store each from its own queue.
                        nc.vector.tensor_scalar_mul(
                            out=o_sb[:, 512:768], in0=op[:, 0:256], scalar1=rsum
                        )
                        nc.scalar.mul(
                            out=o_sb[:, 768:1024], in_=op[:, 256:512], mul=rsum
                        )
                        nc.sync.dma_start(
                            out=out_r[:, st, 512:768], in_=o_sb[:, 512:768]
                        )
                        nc.sync.dma_start(
                            out=out_r[:, st, 768:1024], in_=o_sb[:, 768:1024]
                        )

            pending = emit_s_stage(0)

            # v projection: out[t128, e512], CT tiles stationary
            for tt in range(ST):
                sc, j = tt // 4, tt % 4
                for ec in range(EC):
                    ps = ppool.tile([P, 512], F32, tag="proj")
                    for dt in range(DT):
                        nc.tensor.matmul(
                            ps,
                            ct_sb[:, sc, dt, j * P : (j + 1) * P],
                            wv_sb[:, dt * DIM + ec * 512 : dt * DIM + (ec + 1) * 512],
                            start=(dt == 0),
                            stop=(dt == DT - 1),
                        )
                    nc.vector.tensor_add(
                        out=v_sb[:, tt, ec * 512 : (ec + 1) * 512],
                        in0=ps,
                        in1=bv_sb[:, ec * 512 : (ec + 1) * 512],
                    )

            for st in range(1, ST):
                cur = emit_s_stage(st)
                emit_o_stage(st - 1, *pending)
                pending = cur
            emit_o_stage(ST - 1, *pending)
            attn_cm.__exit__(None, None, None)

    nc.compile()
    return nc


def _w_tiles_flat(w):
    """[1024,1024] -> [p, et*1024+dt*128+j] = w[et*128+j, dt*128+p]."""
    return (
        np.asarray(w, dtype=np.float32)
        .reshape(DT, P, DT, P).transpose(3, 0, 2, 1).reshape(P, DT * DT * P)
    )


def _x_flat(a, bf):  # [2048, 1024] -> [128, 16384]: 2x(dt,256) then 3x(dt,512)
    parts = []
    for s0, w in ((0, 256), (256, 256), (512, 512), (1024, 512), (1536, 512)):
        # [p, dt*w + s] = a[s0+s, dt*128+p]
        parts.append(
            a[s0 : s0 + w].reshape(w, DT, P).transpose(2, 1, 0).reshape(P, DT * w)
        )
    return np.ascontiguousarray(np.concatenate(parts, axis=1)).astype(bf)


def _act_tiles(a, bf):  # [2048, 1024] -> [128, 4, 8, 512]
    return np.ascontiguousarray(
        a.reshape(KC, 512, DT, P).transpose(3, 0, 2, 1)
    ).astype(bf)


def prep_inputs(x, context, Wq, bq, Wk, bk, Wv, bv):
    """Host-side prep for the fast (bq==0) path: M = Wq.T @ Wk folded on the
    host; bk drops (softmax-invariant). Returns per-core input maps."""
    bf = ml_dtypes.bfloat16
    M = np.asarray(Wq, dtype=np.float32).T @ np.asarray(Wk, dtype=np.float32)
    # stationary tile [p, j] must be M[dt*128+p, et*128+j] -> transform of M.T
    mt = np.ascontiguousarray(_w_tiles_flat(M.T)).astype(bf)
    wvt = np.ascontiguousarray(
        np.asarray(Wv, dtype=np.float32).reshape(DIM, DT, P).transpose(2, 1, 0).reshape(P, DT * DIM)
    ).astype(bf)
    bvb = np.ascontiguousarray(
        np.broadcast_to(np.asarray(bv, dtype=np.float32), (P, DIM))
    )
    in_maps = []
    for b in range(B):
        in_maps.append(
            {
                "xc": _x_flat(np.asarray(x[b], dtype=np.float32), bf),
                "cc": _act_tiles(np.asarray(context[b], dtype=np.float32), bf),
                "mt": mt,
                "wvt": wvt,
                "bvb": bvb,
            }
        )
    return in_maps


def build_nc_general():
    nc = bacc.Bacc(None, target_bir_lowering=False)

    # cc: [p, sc, dt, s_lo] = c[sc*512+s_lo, dt*128+p]  (bf16, 8KB lines)
    # xc: flat [p, 16384]; chunks (offset, width, s0): two 256-col chunks
    # first (4KB lines) so the first q matmul needs only 0.5MB of x, then
    # three 512-col chunks (8KB lines).
    xc = nc.declare_dram_parameter("xc", [P, KC * DT * 512], BF16, isOutput=False)
    cc = nc.declare_dram_parameter("cc", [P, KC, DT, 512], BF16, isOutput=False)
    # wq flat: [p, 0:2048]=Wq et0-1 tiles, [p, 2048:2064]=bq/32 || bk (bf16),
    # [p, 2064:4112]=Wq et2-3, [p, 4112:8208]=Wq et4-7. Biases ride the first
    # (small) DMA: a separate 32B-line bias DMA poisons the round-robin share
    # of the queue.
    wqt = nc.declare_dram_parameter("wqt", [P, DT * DT * P + 16], BF16, isOutput=False)
    # wk quarters: [qr, p, e2, dt, j] = Wk[(2qr+e2)*128+j, dt*128+p] (4KB lines)
    wkt = nc.declare_dram_parameter("wkt", [4, P, 2, DT * P], BF16, isOutput=False)
    # wv: [p, dt, e] = Wv[e, dt*128+p]  (16KB lines)
    wvt = nc.declare_dram_parameter("wvt", [P, DT * DIM], BF16, isOutput=False)
    bvb = nc.declare_dram_parameter("bvb", [P, DIM], F32, isOutput=False)
    out = nc.declare_dram_parameter("out", [SEQ, DIM], F32, isOutput=True)

    out_r = out.rearrange("(t p) e -> p t e", p=P)

    with tile.TileContext(nc) as tc:
        with (
            tc.tile_pool(name="resid", bufs=1) as resid,
            tc.tile_pool(name="singles", bufs=1) as singles,
            tc.tile_pool(name="ctpool", bufs=1) as ctpool,
            tc.tile_pool(name="wkpool", bufs=2) as wkpool,
            tc.tile_pool(name="wvpool", bufs=1) as wvpool,
            tc.tile_pool(name="ppool", bufs=3, space="PSUM") as ppool,
            tc.tile_pool(name="spsum", bufs=2, space="PSUM") as spsum,
            tc.tile_pool(name="opsum", bufs=2, space="PSUM") as opsum,
        ):
            qt_sb = resid.tile([P, DT, SEQ], BF16, tag="qt")
            kt_sb = resid.tile([P, DT, SEQ], BF16, tag="kt")
            v_sb = resid.tile([P, ST, DIM], BF16, tag="v")

            bq_sb = singles.tile([P, DT], F32, tag="bq")
            bk_sb = singles.tile([P, DT], F32, tag="bk")
            bv_sb = singles.tile([P, DIM], F32, tag="bv")
            ct_sb = ctpool.tile([P, KC, DT, 512], BF16, tag="ct")
            wv_sb = wvpool.tile([P, DT * DIM], BF16, tag="wv")

            # ---------------- phase 1: projections ----------------
            # xt/wq/warmup live in an inner scope that closes after the q
            # projection, freeing their SBUF + psum bank for the attention
            # pools.
            with (
                tc.tile_pool(name="acts", bufs=1) as acts,
                tc.tile_pool(name="wqpool", bufs=1) as wqpool,
                tc.tile_pool(name="warmps", bufs=1, space="PSUM") as warmps,
            ):
                # Dummy matmuls on a zeroed tile keep the PE busy through the
                # input-DMA window so HAM never sees an idle->busy transition
                # (which costs a ~7-10us half-clock window).
                wsrc = acts.tile([P, 512], BF16, tag="warm")
                nc.vector.memset(wsrc, 0.0)
                wps = warmps.tile([P, 512], F32, tag="wps")
                NWARM = 13
                for i in range(NWARM):
                    nc.tensor.matmul(
                        wps, wsrc[:, 0:P], wsrc, start=(i == 0), stop=(i == NWARM - 1)
                    )

                xt_sb = acts.tile([P, KC * DT * 512], BF16, tag="xt")
                wq_sb = wqpool.tile([P, DT * DT * P + 16], BF16, tag="wq")

                nc.sync.dma_start(out=wq_sb[:, 0:2064], in_=wqt[:, 0:2064])
                nc.sync.dma_start(out=xt_sb[:, 0:2048], in_=xc[:, 0:2048])
                nc.sync.dma_start(out=wq_sb[:, 2064:4112], in_=wqt[:, 2064:4112])
                nc.sync.dma_start(out=wq_sb[:, 4112:6160], in_=wqt[:, 4112:6160])
                nc.sync.dma_start(out=wq_sb[:, 6160:8208], in_=wqt[:, 6160:8208])
                nc.sync.dma_start(out=xt_sb[:, 2048:4096], in_=xc[:, 2048:4096])
                nc.sync.dma_start(out=xt_sb[:, 4096:8192], in_=xc[:, 4096:8192])
                nc.sync.dma_start(out=xt_sb[:, 8192:12288], in_=xc[:, 8192:12288])
                nc.sync.dma_start(out=xt_sb[:, 12288:16384], in_=xc[:, 12288:16384])
                nc.sync.dma_start(out=ct_sb, in_=cc[:, :])
                nc.sync.dma_start(out=wv_sb, in_=wvt[:, :])
                nc.sync.dma_start(out=bv_sb, in_=bvb[:, :])

                nc.vector.tensor_scalar_mul(
                    out=bq_sb, in0=wq_sb[:, 2048:2056], scalar1=1.0
                )
                nc.vector.tensor_scalar_mul(
                    out=bk_sb, in0=wq_sb[:, 2056:2064], scalar1=1.0
                )

                # q projection, chunk-outer in DMA-arrival order: first
                # matmul needs only wq et0-1 + the first 0.5MB of x.
                XCHUNKS = [(0, 256, 0), (2048, 256, 256), (4096, 512, 512),
                           (8192, 512, 1024), (12288, 512, 1536)]
                for xoff, w, s0 in XCHUNKS:
                    for et in range(DT):
                        ps = ppool.tile([P, 512], F32, tag="proj")
                        for dt in range(DT):
                            off = et * DT * P + (16 if et >= 2 else 0) + dt * P
                            nc.tensor.matmul(
                                ps[:, 0:w],
                                wq_sb[:, off : off + P],
                                xt_sb[:, xoff + dt * w : xoff + (dt + 1) * w],
                                start=(dt == 0),
                                stop=(dt == DT - 1),
                            )
                        nc.scalar.activation(
                            out=qt_sb[:, et, s0 : s0 + w],
                            in_=ps[:, 0:w],
                            func=mybir.ActivationFunctionType.Identity,
                            bias=bq_sb[:, et : et + 1],
                            scale=1.0 / 32.0,
                        )

            # k projection, et-outer (ct fully resident by then). wk streams
            # in four 2-et quarters through a bufs=2 pool: each DMA prefetches
            # two ets ahead of consumption, no stall.
            attn_cm = tc.tile_pool(name="attn", bufs=3)
            attn = attn_cm.__enter__()
            stats = attn
            for qr in range(4):
                wk_t = wkpool.tile([P, 2, DT * P], BF16, tag="wk")
                nc.sync.dma_start(out=wk_t, in_=wkt[qr])
                for e4 in range(2):
                    et = 2 * qr + e4
                    for sc in range(KC):
                        ps = ppool.tile([P, 512], F32, tag="proj")
                        for dt in range(DT):
                            nc.tensor.matmul(
                                ps,
                                wk_t[:, e4, dt * P : (dt + 1) * P],
                                ct_sb[:, sc, dt],
                                start=(dt == 0),
                                stop=(dt == DT - 1),
                            )
                        nc.scalar.activation(
                            out=kt_sb[:, et, sc * 512 : (sc + 1) * 512],
                            in_=ps,
                            func=mybir.ActivationFunctionType.Identity,
                            bias=bk_sb[:, et : et + 1],
                            scale=1.0,
                        )

            # ---------------- phase 2: attention ----------------
            # Software-pipelined: S/exp/transpose for tile st is emitted
            # before O/store for tile st-1. S(0) runs right after the k
            # projection; the v projection overlaps S(0)'s exp/transposes.
            def emit_s_stage(st):
                p_sb = attn.tile([P, SEQ], BF16, tag="p")
                pt_sb = attn.tile([P, ST, P], BF16, tag="pt")
                sums = stats.tile([P, KC], F32, tag="sums")
                for kc in range(KC):
                    sp = spsum.tile([P, 512], F32, tag="s")
                    for dt in range(DT):
                        nc.tensor.matmul(
                            sp,
                            qt_sb[:, dt, st * P : (st + 1) * P],
                            kt_sb[:, dt, kc * 512 : (kc + 1) * 512],
                            start=(dt == 0),
                            stop=(dt == DT - 1),
                        )
                    nc.scalar.activation(
                        out=p_sb[:, kc * 512 : (kc + 1) * 512],
                        in_=sp,
                        func=mybir.ActivationFunctionType.Exp,
                        accum_out=sums[:, kc : kc + 1],
                    )
                    # eager xbar transpose of the 4 just-exp'd 128x128
                    # blocks: pt[p, tt, f] = p_sb[f, tt*128 + p]
                    nc.sync.dma_start_transpose(
                        out=pt_sb[:, kc * 4 : (kc + 1) * 4],
                        in_=p_sb[:, kc * 512 : (kc + 1) * 512],
                    )
                ssum = stats.tile([P, 1], F32, tag="ssum")
                rsum = stats.tile([P, 1], F32, tag="rsum")
                nc.vector.reduce_sum(out=ssum, in_=sums, axis=mybir.AxisListType.X)
                nc.vector.reciprocal(out=rsum, in_=ssum)
                return pt_sb, rsum

            def emit_o_stage(st, pt_sb, rsum):
                last = st == ST - 1
                o_sb = attn.tile([P, DIM], F32, tag="o")
                for ec in range(EC):
                    op = opsum.tile([P, 512], F32, tag="o")
                    for tt in range(ST):
                        nc.tensor.matmul(
                            op,
                            pt_sb[:, tt],
                            v_sb[:, tt, ec * 512 : (ec + 1) * 512],
                            start=(tt == 0),
                            stop=(tt == ST - 1),
                        )
                    if not last or ec == 0:
                        nc.vector.tensor_scalar_mul(
                            out=o_sb[:, ec * 512 : (ec + 1) * 512],
                            in0=op,
                            scalar1=rsum,
                        )
                        nc.gpsimd.dma_start(
                            out=out_r[:, st, ec * 512 : (ec + 1) * 512],
                            in_=o_sb[:, ec * 512 : (ec + 1) * 512],
                        )
                    else:
                        # final drain is latency-critical: scale the two
                        # 256-col halves on vector and scalar in parallel,
                        # store each from its own queue.
                        nc.vector.tensor_scalar_mul(
                            out=o_sb[:, 512:768], in0=op[:, 0:256], scalar1=rsum
                        )
                        nc.scalar.mul(
                            out=o_sb[:, 768:1024], in_=op[:, 256:512], mul=rsum
                        )
                        nc.sync.dma_start(
                            out=out_r[:, st, 512:768], in_=o_sb[:, 512:768]
                        )
                        nc.scalar.dma_start(
                            out=out_r[:, st, 768:1024], in_=o_sb[:, 768:1024]
                        )

            pending = emit_s_stage(0)

            # v projection: out[t128, e512], CT tiles stationary
            for tt in range(ST):
                sc, j = tt // 4, tt % 4
                for ec in range(EC):
                    ps = ppool.tile([P, 512], F32, tag="proj")
                    for dt in range(DT):
                        nc.tensor.matmul(
                            ps,
                            ct_sb[:, sc, dt, j * P : (j + 1) * P],
                            wv_sb[:, dt * DIM + ec * 512 : dt * DIM + (ec + 1) * 512],
                            start=(dt == 0),
                            stop=(dt == DT - 1),
                        )
                    nc.vector.tensor_add(
                        out=v_sb[:, tt, ec * 512 : (ec + 1) * 512],
                        in0=ps,
                        in1=bv_sb[:, ec * 512 : (ec + 1) * 512],
                    )

            for st in range(1, ST):
                cur = emit_s_stage(st)
                emit_o_stage(st - 1, *pending)
                pending = cur
            emit_o_stage(ST - 1, *pending)
            stats_cm.__exit__(None, None, None)
            attn_cm.__exit__(None, None, None)

    nc.compile()
    return nc


def prep_inputs_general(x, context, Wq, bq, Wk, bk, Wv, bv):
    """Host-side prep: pre-tiled bf16 activations/weights (contiguous >=4KB
    DMA lines), biases embedded in the wq stream. Returns per-core input maps."""
    bf = ml_dtypes.bfloat16

    def act_tiles(a):  # [2048, 1024] -> [128, 4, 8, 512]
        return np.ascontiguousarray(
            a.reshape(KC, 512, DT, P).transpose(3, 0, 2, 1)
        ).astype(bf)

    def x_flat(a):  # [2048, 1024] -> [128, 16384]: 2x(dt,256) then 3x(dt,512)
        parts = []
        for s0, w in ((0, 256), (256, 256), (512, 512), (1024, 512), (1536, 512)):
            # [p, dt*w + s] = a[s0+s, dt*128+p]
            parts.append(
                a[s0 : s0 + w].reshape(w, DT, P).transpose(2, 1, 0).reshape(P, DT * w)
            )
        return np.ascontiguousarray(np.concatenate(parts, axis=1)).astype(bf)

    # wq tiles [p, et*1024 + dt*128 + j] = Wq[et*128+j, dt*128+p], flattened
    # with the bf16 biases embedded after the first two et tiles.
    wq_tiles = (
        np.asarray(Wq, dtype=np.float32)
        .reshape(DT, P, DT, P).transpose(3, 0, 2, 1).reshape(P, DT * DT * P)
    )
    wqt = np.empty((P, DT * DT * P + 16), dtype=np.float32)
    wqt[:, 0:2048] = wq_tiles[:, 0:2048]
    wqt[:, 2048:2056] = (np.asarray(bq, dtype=np.float32) / 32.0).reshape(DT, P).T
    wqt[:, 2056:2064] = np.asarray(bk, dtype=np.float32).reshape(DT, P).T
    wqt[:, 2064:8208] = wq_tiles[:, 2048:8192]
    wqt = wqt.astype(bf)
    # wk quarters: [qr, p, e2, dt*128+j] = Wk[(2qr+e2)*128+j, dt*128+p]
    wkt = np.ascontiguousarray(
        np.asarray(Wk, dtype=np.float32)
        .reshape(4, 2, P, DT, P).transpose(0, 4, 1, 3, 2).reshape(4, P, 2, DT * P)
    ).astype(bf)
    # [p, dt*1024+e] = Wv[e, dt*128+p]
    wvt = np.ascontiguousarray(
        np.asarray(Wv, dtype=np.float32).reshape(DIM, DT, P).transpose(2, 1, 0).reshape(P, DT * DIM)
    ).astype(bf)
    bvb = np.ascontiguousarray(
        np.broadcast_to(bv.astype(np.float32), (P, DIM))
    )
    in_maps = []
    for b in range(B):
        in_maps.append(
            {
                "xc": x_flat(x[b]),
                "cc": act_tiles(context[b]),
                "wqt": wqt,
                "wkt": wkt,
                "wvt": wvt,
                "bvb": bvb,
            }
        )
    return in_maps




def kernel(x, context, Wq, bq, Wk, bk, Wv, bv):
    global _CACHED_NC, _CACHED_NC_GENERAL
    x = np.asarray(x, dtype=np.float32)
    context = np.asarray(context, dtype=np.float32)
    bq = np.asarray(bq)
    core_ids = list(range(B))
    if not np.any(np.asarray(bq, dtype=np.float32)):
        # fast path: M = Wq.T @ Wk folded on the host; bk is softmax-invariant
        in_maps = prep_inputs(x, context, np.asarray(Wq), bq,
                              np.asarray(Wk), np.asarray(bk),
                              np.asarray(Wv), np.asarray(bv))
        if _CACHED_NC is None:
            _CACHED_NC = build_nc()
        nc = _CACHED_NC
    else:
        in_maps = prep_inputs_general(x, context, np.asarray(Wq), bq,
                                      np.asarray(Wk), np.asarray(bk),
                                      np.asarray(Wv), np.asarray(bv))
        if _CACHED_NC_GENERAL is None:
            _CACHED_NC_GENERAL = build_nc_general()
        nc = _CACHED_NC_GENERAL
    res = run_bass_kernel_spmd(nc, in_maps, core_ids)
    return np.stack([res.results[i]["out"] for i in core_ids]).astype(np.float32)
